# revision 21
# baseline (speedup 1.0000x reference)
"""Trainium2 Bass kernel for a single causal attention head.

Reference (per batch element b):
    q = x[b] @ Wq; k = x[b] @ Wk; v = x[b] @ Wv          # [T, HD]
    S = q @ k.T;  S = where(tril, S, -inf) / sqrt(C)
    out[b] = softmax(S, -1) @ v                           # [T, HD]

Sharding: pure data parallel -- core i computes batch element i
(B == 8 == n_cores). No collectives.

Device algorithm (per core), v4:
  * x streams in fp8: x8 = e4m3(xT) plus a residual xr8 = e4m3(xT - x8),
    both in DoubleRow layout [64, ct, 2, t] (c split into two 64-halves),
    one DMA per 512-column t-chunk. Same total bytes as bf16 but the
    first chunk lands in ~1.5us, so the exp chain starts ~4us in.
  * projections run as fp8 DoubleRow matmuls (2 rows/cycle): q,k from x8
    alone (quantization only perturbs softmax logits ~0.5%); v
    accumulates BOTH passes x8@Wv + xr8@Wv in one PSUM group, restoring
    ~bf16 accuracy where it matters. Weights are pre-scaled by 32 (power
    of two) into fp8's dynamic range; the 1/32 folds into the exp scale
    for q,k and cancels via the ones-column (set to 32) for v.
  * scores are computed TRANSPOSED in bf16: S_T[s, t] = kT_slice.T @ qT.
    Two s-blocks share one 2-bank PSUM tile and ONE ScalarE exp call.
    Causal masking = one constant [128,512] triangular tile multiplied on
    VectorE (diagonal blocks only); fully masked blocks are never
    computed, partially masked blocks are column-trimmed (the odd half of
    a diagonal pair computes untrimmed so the merged exp range is
    contiguously written; its sub-diagonal garbage is never read).
  * attv is computed in NATURAL output layout per 128-row t-tile:
    stationary = es [128s, 128t] slice, moving = [v | 32s] so one PSUM
    tile accumulates out_unnorm[t, d] AND the softmax row-sums (col 64).
    attv t-tiles are woven BETWEEN score pairs so the PE always has
    Act-independent work; VectorE takes a reciprocal of col 64 and scales
    cols 0..63; each 128-row block DMAs out in bf16 (host casts to f32).
  * k/v PSUM->SBUF copies run on GpSimd, q copies + masks + normalize on
    VectorE, exp on ScalarE: all engines stream concurrently.
"""

import numpy as np

B, T, C, HD = 8, 2048, 1024, 64
NCORES = 8
CHUNK = 512
NJ = T // CHUNK
NCT = C // 128
NST = T // 128
SCALE = 1.0 / np.sqrt(np.float32(C))
WSC = 32.0                  # weight pre-scale (power of two)

MODE = "bf16"
WARMUP_MM = 30
WARMUP_N = 32


def build_bass(mode=MODE, reps=1):
    import concourse.bacc as bacc
    import concourse.tile as tile
    import concourse.mybir as mybir

    f32 = mybir.dt.float32
    st_dt = mybir.dt.bfloat16
    f8 = mybir.dt.float8e4

    EXP = mybir.ActivationFunctionType.Exp
    GE = mybir.AluOpType.is_ge
    DR = mybir.MatmulPerfMode.DoubleRow

    nc = bacc.Bacc("TRN2", target_bir_lowering=False, debug=False,
                   num_devices=NCORES)
    # DoubleRow layout: contraction c packed as 4 tiles of (128
    # partitions x 2 slices) = 256-deep fp8 matmul tiles, all full-width
    # on partitions (both for the DMA engines and the PE)
    x8d = nc.dram_tensor("x8", [NJ, 128, 2, NCT // 2, 2, CHUNK // 2], f8,
                         kind="ExternalInput")
    xr8d = nc.dram_tensor("xr8", [NJ, 128, 2, NCT // 2, 2, CHUNK // 2],
                          f8, kind="ExternalInput")
    w8qkd = nc.dram_tensor("w8qk", [128, NCT // 2, 2, 128], f8,
                           kind="ExternalInput")
    w8vd = nc.dram_tensor("w8v", [128, NCT // 2, 2, 128], f8,
                          kind="ExternalInput")
    out = nc.dram_tensor("out", [T, HD], st_dt, kind="ExternalOutput")

    with tile.TileContext(nc) as tc:
        with (
            tc.tile_pool(name="consts", bufs=1) as consts,
            tc.tile_pool(name="xin8", bufs=NJ) as xin8,
            tc.tile_pool(name="xinr", bufs=NJ) as xinr,
            tc.tile_pool(name="proj", bufs=1) as proj,
            tc.tile_pool(name="es", bufs=21) as es_pool,
            tc.tile_pool(name="small", bufs=4) as small,
            tc.tile_pool(name="psA", bufs=2, space="PSUM") as psA,
            tc.tile_pool(name="psQK", bufs=2, space="PSUM") as psQK,
            tc.tile_pool(name="psV", bufs=2, space="PSUM") as psV,
        ):
            # PE warmup source: zeroed by DVE so PE can start ~immediately,
            # keeping the HAM clock-gate warm while input DMAs stream in.
            warm_src = consts.tile([128, WARMUP_N], st_dt, tag="warm")
            nc.vector.memset(warm_src[:], 0.0)
            warm_ps = psV.tile([128, WARMUP_N], f32, tag="v")
            for _w in range(WARMUP_MM):
                nc.tensor.matmul(warm_ps[0:WARMUP_N, :], warm_src[:],
                                 warm_src[:], start=True, stop=True)

            # qk weights first: the first projection needs only them;
            # v weights stream after the x8 chunks (v runs late anyway)
            w8qk_sb = consts.tile([128, NCT // 2, 2, 128], f8, tag="wqk")
            nc.sync.dma_start(w8qk_sb[:], w8qkd[:, :, :, :])
            w8v_sb = consts.tile([128, NCT // 2, 2, 128], f8, tag="wv")

            # causal mask M[s, y] = 1 if y >= s else 0  (shared by all
            # diagonal blocks; diagonal block r uses M[:, 0:512-128r])
            cmask = consts.tile([128, CHUNK], st_dt, tag="cmask")
            nc.gpsimd.memset(cmask[:], 1.0)
            nc.gpsimd.affine_select(
                out=cmask[:], in_=cmask[:], compare_op=GE, fill=0.0,
                base=0, channel_multiplier=-1, pattern=[[1, CHUNK]],
            )

            for _rep in range(reps):
                emit_body(nc, tc, st_dt, f32, f8, EXP, DR, cmask,
                          w8qk_sb, w8v_sb, proj, xin8, xinr, es_pool, small,
                          psA, psQK, psV, x8d, xr8d, w8vd, out)

    nc.compile()
    return nc


def emit_body(nc, tc, st_dt, f32, f8, EXP, DR, cmask, w8qk_sb, w8v_sb,
              proj, xin8, xinr, es_pool, small, psA, psQK, psV,
              x8d, xr8d, w8vd, out):
    q_sb = proj.tile([64, T], st_dt, tag="q")
    k_sb = proj.tile([64, T], st_dt, tag="k")
    v65 = proj.tile([128, NST * 65], st_dt, tag="v65")
    for st in range(NST):
        # ones-column = 32 cancels the 1/32 carried by the fp8-scaled Wv
        nc.gpsimd.memset(v65[:, st * 65 + 64: st * 65 + 65], WSC)

    x8s, xr8s = {}, {}
    for j in range(NJ):
        x8t = xin8.tile([128, 2, NCT // 2, 2, CHUNK // 2], f8, tag="x8")
        # two half-chunk DMAs so the first q,k projection starts sooner
        nc.sync.dma_start(x8t[:, 0], x8d[j, :, 0])
        nc.sync.dma_start(x8t[:, 1], x8d[j, :, 1])
        x8s[j] = x8t
    nc.sync.dma_start(w8v_sb[:], w8vd[:, :, :, :])
    for j in range(NJ):
        xr8t = xinr.tile([128, 2, NCT // 2, 2, CHUNK // 2], f8, tag="xr8")
        nc.sync.dma_start(xr8t[:], xr8d[j, :, :, :, :, :])
        xr8s[j] = xr8t

    ess = {}

    def emit_qk(j):
        for uh in (0, 1):
            ps_qk = psQK.tile([128, CHUNK], f32, tag="qk")
            for i in range(NCT // 2):
                nc.tensor.matmul(
                    ps_qk[:, 0:CHUNK // 2],
                    w8qk_sb[:, i, :, :],
                    x8s[j][:, uh, i, :, :],
                    start=(i == 0), stop=(i == NCT // 2 - 1),
                    perf_mode=DR,
                )
            t0 = j * CHUNK + uh * (CHUNK // 2)
            nc.vector.tensor_copy(q_sb[:, t0:t0 + CHUNK // 2],
                                  ps_qk[0:64, 0:CHUNK // 2])
            nc.vector.tensor_copy(k_sb[:, t0:t0 + CHUNK // 2],
                                  ps_qk[64:128, 0:CHUNK // 2])

    def emit_v(j):
        # three fp8 passes accumulate into one PSUM group:
        # v = x8@Wv8 + x8@Wv8r + xr8@Wv8  (the dropped xr8@Wv8r cross term
        # is ~0.1%); both x and W quantization are residual-compensated so
        # v reaches ~bf16 accuracy
        for r in range(4):
            st = 4 * j + r
            uh, u0 = r // 2, (r % 2) * 128
            ps_v = psV.tile([128, HD], f32, tag="v")
            first = True
            for xsrc, wlo in ((x8s[j], 0), (x8s[j], 64), (xr8s[j], 0)):
                for i in range(NCT // 2):
                    nc.tensor.matmul(
                        ps_v[:],
                        xsrc[:, uh, i, :, u0:u0 + 128],
                        w8v_sb[:, i, :, wlo:wlo + HD],
                        start=first, stop=(wlo == 0 and xsrc is xr8s[j]
                                           and i == NCT // 2 - 1),
                        perf_mode=DR,
                    )
                    first = False
            nc.vector.tensor_copy(v65[:, st * 65: st * 65 + HD], ps_v[:, :])

    def emit_scores_pair(j, st0):
        # two s-blocks (st0, st0+1) share one 2-bank PSUM tile and ONE
        # ScalarE exp (the fp8 weight pre-scale 32^2 divides out here)
        ps = psA.tile([128, 2 * CHUNK], f32, tag="mm")
        es = es_pool.tile([128, 2 * CHUNK], st_dt, tag="es")
        offs = []
        for h in (0, 1):
            st = st0 + h
            r = st - 4 * j           # >=0 on diagonal tiles
            off = 128 * r if r > 0 else 0
            nc.tensor.matmul(
                ps[:, h * CHUNK + off: (h + 1) * CHUNK],
                k_sb[:, st * 128:(st + 1) * 128],
                q_sb[:, j * CHUNK + off:(j + 1) * CHUNK],
                start=True, stop=True,
            )
            ess[j, st] = (es, h * CHUNK)
            offs.append(off)
        act_scale = float(SCALE / (WSC * WSC))
        if offs[1] == 0:
            nc.scalar.activation(es[:, offs[0]:2 * CHUNK],
                                 ps[:, offs[0]:2 * CHUNK], EXP,
                                 scale=act_scale)
        else:
            # diagonal pair: two trimmed calls skip the inter-block gap
            nc.scalar.activation(es[:, offs[0]:CHUNK],
                                 ps[:, offs[0]:CHUNK], EXP, scale=act_scale)
            nc.scalar.activation(es[:, CHUNK + offs[1]:2 * CHUNK],
                                 ps[:, CHUNK + offs[1]:2 * CHUNK], EXP,
                                 scale=act_scale)
        for h in (0, 1):
            st = st0 + h
            r = st - 4 * j
            if r >= 0:
                off = 128 * r if r > 0 else 0
                n = CHUNK - off
                # zero the sub-diagonal half: es[s, y] *= (y >= s)
                # (on GpSimd: SBUF-only op, keeps VectorE free for copies)
                nc.gpsimd.tensor_mul(es[:, h * CHUNK + off:(h + 1) * CHUNK],
                                     es[:, h * CHUNK + off:(h + 1) * CHUNK],
                                     cmask[:, 0:n])

    attv_open = {}

    def attv_part(j, kk, st_end):
        # natural-layout accumulation for 128-row t-tile tt = 4j + kk:
        # stationary = es [128s, 128t] slice, moving = [v | 32s]; PSUM
        # collects out_unnorm[t, 0:64] and 32*row-sums in col 64.
        tt = 4 * j + kk
        if (j, kk) in attv_open:
            ps_o, st_from = attv_open[j, kk]
        else:
            ps_o, st_from = psV.tile([128, 65], f32, tag="v"), 0
        for st in range(st_from, st_end + 1):
            es, base = ess[j, st]
            nc.tensor.matmul(
                ps_o[:, 0:65],
                es[:, base + kk * 128:base + (kk + 1) * 128],
                v65[:, st * 65:(st + 1) * 65],
                start=(st == 0), stop=(st == tt),
            )
        if st_end < tt:
            attv_open[j, kk] = (ps_o, st_end + 1)
            return
        attv_open.pop((j, kk), None)
        ob = small.tile([128, HD], st_dt, tag="ob")
        rec = small.tile([128, 1], f32, tag="rec")
        nc.vector.reciprocal(rec[:], ps_o[:, 64:65])
        nc.vector.tensor_scalar_mul(ob[:], ps_o[:, 0:HD], rec[:])
        nc.sync.dma_start(out[tt * 128:(tt + 1) * 128, :], ob[:])

    def emit_attv_tile(j, kk):
        attv_part(j, kk, 4 * j + kk)

    emit_qk(0)
    emit_scores_pair(0, 0)
    emit_scores_pair(0, 2)
    emit_qk(1)
    emit_scores_pair(1, 0)
    emit_scores_pair(1, 2)
    emit_scores_pair(1, 4)
    emit_scores_pair(1, 6)
    emit_qk(2)
    emit_scores_pair(2, 0)
    emit_scores_pair(2, 2)
    emit_scores_pair(2, 4)
    emit_scores_pair(2, 6)
    emit_scores_pair(2, 8)
    emit_scores_pair(2, 10)
    emit_qk(3)
    emit_v(0)
    emit_scores_pair(3, 0)
    emit_scores_pair(3, 2)
    emit_attv_tile(0, 0)
    emit_scores_pair(3, 4)
    emit_attv_tile(0, 1)
    emit_v(1)
    emit_scores_pair(3, 6)
    emit_attv_tile(0, 2)
    emit_attv_tile(0, 3)
    emit_scores_pair(3, 8)
    emit_attv_tile(1, 0)
    emit_attv_tile(1, 1)
    emit_v(2)
    emit_scores_pair(3, 10)
    emit_attv_tile(1, 2)
    emit_attv_tile(1, 3)
    emit_scores_pair(3, 12)
    emit_attv_tile(2, 0)
    emit_attv_tile(2, 1)
    emit_v(3)
    emit_attv_tile(2, 2)
    emit_attv_tile(2, 3)
    emit_attv_tile(3, 0)
    emit_attv_tile(3, 1)
    attv_part(3, 2, 13)
    attv_part(3, 3, 13)
    emit_scores_pair(3, 14)
    attv_part(3, 2, 14)
    attv_part(3, 3, 15)


def prep_inputs(x, Wq, Wk, Wv, mode=MODE):
    import ml_dtypes

    f8 = ml_dtypes.float8_e4m3
    x = np.asarray(x, dtype=np.float32)

    wqk = np.concatenate([np.asarray(Wq), np.asarray(Wk)], axis=1)  # [C,128]
    w8qk = (WSC * wqk).reshape(NCT // 2, 2, 128, 128).transpose(
        2, 0, 1, 3).astype(f8)
    wv = (WSC * np.asarray(Wv)).astype(np.float32)
    wv8 = wv.astype(f8)
    wv8r = (wv - wv8.astype(np.float32)).astype(f8).astype(np.float32)
    lay_w = lambda a: np.asarray(a, np.float32).reshape(
        NCT // 2, 2, 128, HD).transpose(2, 0, 1, 3)
    w8v = np.concatenate([lay_w(wv8), lay_w(wv8r)], axis=3).astype(f8)

    in_maps = []
    for b in range(NCORES):
        xT = np.ascontiguousarray(x[b].T)                 # [C, T]
        x8 = xT.astype(f8)
        xr8 = (xT - x8.astype(np.float32)).astype(f8)
        def lay(a):
            # [NJ, 128p, uh2, ct4, 2slot, 256]: c = ct*256 + slot*128 + p,
            # t = j*512 + uh*256 + u
            a = a.reshape(NCT // 2, 2, 128, NJ, 2, CHUNK // 2)
            return np.ascontiguousarray(a.transpose(3, 2, 4, 0, 1, 5))
        in_maps.append({"x8": lay(x8), "xr8": lay(xr8),
                        "w8qk": w8qk, "w8v": w8v})
    return in_maps


_NC_CACHE = {}


def kernel(x, Wq, Wk, Wv):
    from concourse.bass_utils import run_bass_kernel_spmd

    if MODE not in _NC_CACHE:
        _NC_CACHE[MODE] = build_bass(MODE)
    nc = _NC_CACHE[MODE]
    in_maps = prep_inputs(np.asarray(x), np.asarray(Wq), np.asarray(Wk),
                          np.asarray(Wv), MODE)
    res = run_bass_kernel_spmd(nc, in_maps, core_ids=list(range(NCORES)))
    return np.stack([np.asarray(res.results[b]["out"]).astype(np.float32)
                     for b in range(NCORES)], axis=0)
